# revision 31
# baseline (speedup 1.0000x reference)
"""AutoCorrelation Trainium2 kernel (Bass/Tile, 8 NeuronCores) — v2.

Math (per row r of [B*L, 512] with D=512):
  corr_r = irfft(rfft(q_r) * conj(rfft(k_r)))            (circular cross-correlation)
  mean_r = mean(top7(corr_r))
  out = v + sigmoid(mean - corr) * (roll(v,-1,L) - v)

Implementation notes:
  - Host casts q/k/v to fp16 before upload and the kernel emits an fp16
    output (cast back to fp32 on host): HBM traffic 33 MiB/core instead of
    64, and every DMA is HWDGE (no SWDGE casts) so GpSimd is free for
    elementwise work.
  - DFT/iDFT as fp16 matmuls with a packed-real 512-point basis
    (A-block f=0..255 = Re[f] with A[0]=Re0, B-block = Im[f] with
    B[0]=Re256).  Forward rhs comes from an xbar DMA-transpose pulled
    STRAIGHT from DRAM.  q/k share each W-block LDWEIGHTS (interleaved).
  - Product spectrum on DVE fp16 (2x_1P mode, no Pool-port contention),
    superblock-wide FD=2048 ops + one f=0 fixup op.
  - Inverse GEMM accumulates into PSUM with C pre-scaled by 1/7, so
    reduce_sum(top7) IS the top-k mean; ACT sigmoid(bias=mean, scale=-7)
    reads PSUM directly.
  - Row interleave: partition p = row//64, subblock s = row%64 so
    roll(v,-1) = "read subblock s+1"; v is loaded 9 subblocks per
    8-subblock superblock (vnext = v9[:,1:9]); the last superblock's 9th
    slot is filled by a strided row-64k load + 4 batch-wrap rows.
  - Sharding: batch-parallel, 4 batches per core, no communication.
"""
import numpy as np

B, L, D = 32, 2048, 512
N_CORES = 8
BPC = B // N_CORES            # batches per core
ROWS = BPC * L                # 8192 rows per core
NSUB = 64                     # subblocks (s = row % 64)
P = 128                       # partitions (p = row // 64)
SB_GROUP = 8                  # subblocks per superblock
NSUPER = NSUB // SB_GROUP     # 8 superblocks
TOPK = 7

_CACHE = {}


def _dft_consts():
    """Packed-real DFT matrices W [512 feat, 512 packed] and C [512 packed, 512 t].
    C is pre-scaled by 1/TOPK so sum(top7(corr')) == mean(top7(corr))."""
    j = np.arange(D)[:, None].astype(np.float64)
    f = np.arange(256)[None, :].astype(np.float64)
    Wc = np.cos(-2 * np.pi * j * f / D)
    Ws = np.sin(-2 * np.pi * j * f / D)
    WB = Ws.copy()
    WB[:, 0] = np.cos(np.pi * j[:, 0])          # B0 row: Re256
    W = np.concatenate([Wc, WB], axis=1)        # [512, 512]
    t = np.arange(D)[None, :].astype(np.float64)
    fc = np.arange(256)[:, None].astype(np.float64)
    Ca = np.cos(2 * np.pi * fc * t / D) * 2 / D
    Ca[0] = 1.0 / D
    Cb = -np.sin(2 * np.pi * fc * t / D) * 2 / D
    Cb[0] = np.cos(np.pi * t[0]) / D
    C = np.concatenate([Ca, Cb], axis=0) / TOPK  # [512, 512]
    return W.astype(np.float32), C.astype(np.float32)


def _build_nc(n_iter=1, n_super=NSUPER):
    import concourse.bacc as bacc
    import concourse.mybir as mybir
    from concourse.tile import TileContext

    f16 = mybir.dt.float16
    f32 = mybir.dt.float32

    W, C = _dft_consts()
    # W16[p, jj, fp]  = W[jj*128+p, fp]   (lhsT blocks for GEMM-1)
    W16 = W.reshape(4, P, D).transpose(1, 0, 2).astype(np.float16).copy()
    # C16[p, ff, t]   = C[ff*128+p, t]    (rhs blocks for GEMM-2)
    C16 = C.reshape(4, P, D).transpose(1, 0, 2).astype(np.float16).copy()

    nc = bacc.Bacc()
    # query/key arrive HOST-PRE-TRANSPOSED as [a, sbi, jj, s, r]:
    # element = q[row 64*r + 8*sbi + s, feat jj*128+a].  Each superblock's
    # slice [:, sbi] is one contiguous 8 KiB/partition block, so the
    # forward-DFT rhs is plain full-rate DMA loads (no xbar transposes).
    q_d = nc.dram_tensor("query", [P, NSUPER, 4, SB_GROUP, P], f16,
                         kind="ExternalInput")
    k_d = nc.dram_tensor("key", [P, NSUPER, 4, SB_GROUP, P], f16,
                         kind="ExternalInput")
    v_d = nc.dram_tensor("value", [ROWS, D], f16, kind="ExternalInput")
    o_d = nc.dram_tensor("out", [ROWS, D], f16, kind="ExternalOutput")
    w_t = nc.inline_tensor(W16, name="Wdft")
    c_t = nc.inline_tensor(C16, name="Cdft")

    # interleaved views: [p, s, c] with row = 64*p + s
    vv = v_d.rearrange("(p s) c -> p s c", s=NSUB)
    ov = o_d.rearrange("(p s) c -> p s c", s=NSUB)

    with TileContext(nc) as tc:
        with (
            tc.tile_pool(name="consts", bufs=1) as consts,
            tc.tile_pool(name="io", bufs=2) as io,
            tc.tile_pool(name="work", bufs=2) as work,
            tc.tile_pool(name="small", bufs=8) as small,
            tc.tile_pool(name="ps", bufs=3, space="PSUM") as psp,
            tc.tile_pool(name="pscb", bufs=2, space="PSUM") as pscp,
        ):
            wt = consts.tile([P, 4, D], f16)      # W16
            ct = consts.tile([P, 4, D], f16)      # C16
            nc.sync.dma_start(out=wt, in_=w_t[:, :, :])
            # ct is loaded inside superblock 0, after its input DMAs: the
            # inverse GEMM doesn't need it until ~20us in, and this keeps the
            # first q/k transposes at the head of the SP ring.

            # dummy sigmoid: forces the one ACT table-set load into the fill
            # window (otherwise it stalls the pipe at the first real sigmoid)
            warm = small.tile([1, 8], f16, tag="warm")
            nc.scalar.activation(warm, wt[0:1, 0, 0:8],
                                 mybir.ActivationFunctionType.Sigmoid,
                                 scale=-1.0)

            def superblock(sbi, pending_store):
                sl = slice(sbi * SB_GROUP, (sbi + 1) * SB_GROUP)
                # qT8[a, jj, s, r] = q[row 64r + 8sbi + s, jj*128+a]
                qT8 = work.tile([P, 4, SB_GROUP, P], f16, tag="qT8")
                kT8 = work.tile([P, 4, SB_GROUP, P], f16, tag="kT8")
                for jh in range(2):
                    js = slice(2 * jh, 2 * jh + 2)
                    nc.sync.dma_start(out=qT8[:, js, :, :],
                                      in_=q_d[:, sbi, js, :, :])
                    nc.sync.dma_start(out=kT8[:, js, :, :],
                                      in_=k_d[:, sbi, js, :, :])

                v9 = io.tile([P, SB_GROUP + 1, D], f16, tag="v9")
                if sbi < NSUPER - 1:
                    nc.sync.dma_start(
                        out=v9, in_=vv[:, sbi * SB_GROUP:(sbi + 1) * SB_GROUP + 1, :])
                else:
                    nc.sync.dma_start(out=v9[:, 0:SB_GROUP, :], in_=vv[:, sl, :])
                    # v9[p, 8] = v[row 64p+64]; wraps at p in {31,63,95,127}
                    nc.sync.dma_start(
                        out=v9[0:127, SB_GROUP, :],
                        in_=v_d.rearrange("(a b) c -> a b c", b=NSUB)[1:128, 0])
                    nc.sync.dma_start(
                        out=v9.rearrange("(w u) s c -> w u s c", u=32)[:, 31, SB_GROUP, :],
                        in_=v_d.rearrange("(b t) c -> b t c", t=L)[:, 0, :])

                # previous superblock's store, issued AFTER this one's loads so
                # the SP ring services loads first (FIFO per ring)
                if pending_store is not None:
                    po16, psl = pending_store
                    nc.sync.dma_start(out=ov[:, psl, :], in_=po16)
                if sbi == 0 and n_iter == 1:
                    nc.sync.dma_start(out=ct, in_=c_t[:, :, :])

                # forward DFT: psq/psk [freq-chunk mm, rows], q/k share LDWEIGHTS
                qf = work.tile([P, 4, 4 * 256], f16, tag="qf")
                kf = work.tile([P, 4, 4 * 256], f16, tag="kf")
                for gl in range(4):
                    psq = psp.tile([P, 4, 256], f32, tag="ps2bank")
                    psk = psp.tile([P, 4, 256], f32, tag="ps2bank")
                    for mm in range(4):
                        for jj in range(4):
                            lw = wt[:, jj, mm * P:(mm + 1) * P]
                            rq = qT8[:, jj, 2 * gl:2 * gl + 2, :]
                            rk = kT8[:, jj, 2 * gl:2 * gl + 2, :]
                            nc.tensor.matmul(psq[:, mm, :], lw, rq,
                                             start=(jj == 0), stop=(jj == 3))
                            nc.tensor.matmul(psk[:, mm, :], lw, rk,
                                             start=(jj == 0), stop=(jj == 3))
                    nc.scalar.copy(qf[:, :, gl * 256:(gl + 1) * 256], psq)
                    nc.scalar.copy(kf[:, :, gl * 256:(gl + 1) * 256], psk)

                # product spectrum P = QF o conj(KF) on DVE, in two row-halves
                # so the inverse GEMM of half 0 overlaps the half-1 products
                pt = work.tile([P, 4, 1024], f16, tag="pt")
                t1 = work.tile([P, 2, 1024], f16, tag="t1")
                t2 = work.tile([P, 2, 1024], f16, tag="t2")
                for rh in range(2):
                    rs = slice(rh * 512, (rh + 1) * 512)
                    QA, QB = qf[:, 0:2, rs], qf[:, 2:4, rs]
                    KA, KB = kf[:, 0:2, rs], kf[:, 2:4, rs]
                    T1, T2 = t1[:, :, rs], t2[:, :, rs]
                    nc.vector.tensor_mul(T1, QA, KA)
                    nc.vector.tensor_mul(T2, QB, KB)
                    nc.vector.tensor_add(pt[:, 0:2, rs], T1, T2)
                    nc.vector.tensor_mul(T1, QB, KA)
                    nc.vector.tensor_mul(T2, QA, KB)
                    nc.vector.tensor_sub(pt[:, 2:4, rs], T1, T2)
                    # f=0 fixup (partition 0 of slices 0 and 2)
                    nc.vector.tensor_mul(
                        pt[0:1, 0:4:2, rs], qf[0:1, 0:4:2, rs], kf[0:1, 0:4:2, rs])

                # inverse DFT per 128-row chunk (= subblock), then w1 weights
                w1sb = work.tile([P, SB_GROUP, D], f16, tag="w1sb")
                for ch in range(SB_GROUP):
                    cps = pscp.tile([P, D], f32, tag="psc1bank")
                    for ff in range(4):
                        nc.tensor.matmul(cps, pt[:, ff, ch * P:(ch + 1) * P],
                                         ct[:, ff, :], start=(ff == 0), stop=(ff == 3))
                    mx = small.tile([P, 8], f32, tag="mx")
                    nc.vector.max(out=mx, in_=cps)
                    pm = small.tile([P, 1], f32, tag="pm")
                    nc.vector.reduce_sum(pm, mx[:, 0:TOPK],
                                         axis=mybir.AxisListType.X)
                    nc.scalar.activation(w1sb[:, ch, :], cps,
                                         mybir.ActivationFunctionType.Sigmoid,
                                         bias=pm, scale=-float(TOPK))

                # out = v + w1*(vnext - v), all on GpSimd, in two halves so the
                # first half overlaps the second half's inverse/sigmoid chain
                dt_ = work.tile([P, SB_GROUP, D], f16, tag="dt")
                zt = work.tile([P, SB_GROUP, D], f16, tag="zt")
                o16 = io.tile([P, SB_GROUP, D], f16, tag="o16")
                H = SB_GROUP // 2
                for h in range(2):
                    hs = slice(h * H, (h + 1) * H)
                    nc.gpsimd.tensor_sub(dt_[:, hs, :],
                                         v9[:, h * H + 1:(h + 1) * H + 1, :],
                                         v9[:, hs, :])
                    nc.vector.tensor_mul(zt[:, hs, :], w1sb[:, hs, :], dt_[:, hs, :])
                    nc.gpsimd.tensor_add(o16[:, hs, :], v9[:, hs, :], zt[:, hs, :])
                return o16, sl

            def pipeline():
                pending = None
                for sbi in range(n_super):
                    pending = superblock(sbi, pending)
                po16, psl = pending
                nc.sync.dma_start(out=ov[:, psl, :], in_=po16)

            if n_iter == 1:
                pipeline()
            else:
                nc.sync.dma_start(out=ct, in_=c_t[:, :, :])
                with tc.For_i(0, n_iter, 1):
                    pipeline()

    nc.finalize()
    return nc


def kernel(query, key, value):
    import sys
    if "/opt/trn_rl_repo" not in sys.path:
        sys.path.insert(0, "/opt/trn_rl_repo")
    from concourse.bass_utils import run_bass_kernel_spmd

    if "nc" not in _CACHE:
        _CACHE["nc"] = _build_nc()
    nc = _CACHE["nc"]

    q = np.asarray(query, dtype=np.float32).reshape(B, L, D).astype(np.float16)
    k = np.asarray(key, dtype=np.float32).reshape(B, L, D).astype(np.float16)
    v = np.asarray(value, dtype=np.float32).reshape(B, L, D).astype(np.float16)

    def pre_t(x, c):
        # [ROWS, D] -> [a, sbi, jj, s, r]: out = x[64*r + 8*sbi + s, jj*128+a]
        xc = x[c * BPC:(c + 1) * BPC].reshape(ROWS, D)
        return np.ascontiguousarray(
            xc.reshape(P, NSUPER, SB_GROUP, 4, P).transpose(4, 1, 3, 2, 0))

    in_maps = []
    for c in range(N_CORES):
        sl = slice(c * BPC, (c + 1) * BPC)
        in_maps.append({
            "query": pre_t(q, c),
            "key": pre_t(k, c),
            "value": np.ascontiguousarray(v[sl].reshape(ROWS, D)),
        })
    res = run_bass_kernel_spmd(nc, in_maps, core_ids=list(range(N_CORES)),
                               trace=bool(_CACHE.get("trace")))
    _CACHE["last_result"] = res
    out = np.empty((B, L, D), dtype=np.float32)
    for c in range(N_CORES):
        out[c * BPC:(c + 1) * BPC] = res.results[c]["out"].astype(
            np.float32).reshape(BPC, L, D)
    return out
